# revision 1
# baseline (speedup 1.0000x reference)
"""Trainium2 Bass kernel for BinarizeConv2d block:
   y = round(2*clip(BN(conv3x3(x, sign(w))), -1, 1))/2

Output-channel sharding: each of 8 cores computes 4 output channels for ALL
16 images, so BN batch stats are fully local and NO collective is needed
(a collective would couple every core's NEFF span to the slowest core's
host->device staging, which dominates single-dispatch time).

Conv: x is shipped once as fp16 (exact products with +-1 weights; rel err
~1e-2 vs 2e-2 budget). 16 images run as 4 partition-lanes (g) x 4 batches.
K=32 (ci) matmuls on 16 concurrent 32x32 PE tiles: rows=32g (image lane),
cols=32j. Each col tile packs 28 row-pairs x 4 channels via zero-padded
weight columns: chain (j,k) has its 4 real weight cols at 4k+m and zeros
elsewhere, so every matmul writes the full 32-col group; the first chain
clears (start=True), later chains accumulate zeros harmlessly. PSUM comes
out dense -> full-width ACT drains, no repack.

Epilogue: bn_stats over the dense y_raw, cross-partition combine via tiny
fp32 sel matmuls, Newton-polished rsqrt, ACT affine + DVE/GPSIMD magic-number
round/clip to bf16 {-2..2}; host multiplies by 0.5 and concatenates the
8 cores' channel groups.
"""
import sys
sys.path.insert(0, "/opt/trn_rl_repo")
import numpy as np
import ml_dtypes
import concourse.bass as bass
import concourse.bacc as bacc
import concourse.tile as tile
from concourse import mybir
from concourse.bass_utils import run_bass_kernel_spmd

F32 = mybir.dt.float32
F16 = mybir.dt.float16
BF16 = mybir.dt.bfloat16

N_CORES = 8
CO_PC = 4         # output channels per core
C = 32
H = W = 224
WP = 226          # padded width
NB = 4            # image batches (4 lanes each)
NS = 4            # 56-row supers per image
SR = 56           # rows per super
SLOTS = 58        # input rows per super window (56 + 2 halo)
MAGIC = 12582912.0  # 1.5 * 2**23 -> fp32 round-to-nearest-even trick
EPS = 1e-5
NTOT = float(16 * H * W)  # elements per channel (all on one core)
HWs = H * W

_cache = {}


def _build_nc(loop_n=1, skip=(), dbg=False):
    nc = bacc.Bacc("TRN2", target_bir_lowering=False, debug=False,
                   num_devices=N_CORES)
    xs_ext = nc.declare_dram_parameter("xs", [16, C, H + 2, WP], F16,
                                       isOutput=False)
    dbg_ext = (nc.declare_dram_parameter("dbg", [128, NB, NS, 4, 448], F32,
                                         isOutput=True) if dbg else None)
    dbg2_ext = (nc.declare_dram_parameter("dbg2", [128, 8], F32,
                                          isOutput=True) if dbg == 2 else None)
    s_ext = nc.declare_dram_parameter("s", [128, 9, 7, 32], F16, isOutput=False)
    sel1_ext = nc.declare_dram_parameter("sel1", [128, CO_PC], F32,
                                         isOutput=False)
    sel2_ext = nc.declare_dram_parameter("sel2", [CO_PC, 128], F32,
                                         isOutput=False)
    g_ext = nc.declare_dram_parameter("g", [CO_PC, 1], F32, isOutput=False)
    b_ext = nc.declare_dram_parameter("b", [CO_PC, 1], F32, isOutput=False)
    y_ext = nc.declare_dram_parameter("y", [16, CO_PC, H, W], BF16,
                                      isOutput=True)

    with tile.TileContext(nc) as tc:
        with (
            tc.tile_pool(name="big", bufs=1) as big,
            tc.tile_pool(name="small", bufs=1) as small,
            tc.tile_pool(name="ph2", bufs=2) as ph2,
            tc.tile_pool(name="psum", bufs=1, space="PSUM") as psum,
        ):
            # x super chunk: partition p = 32g + ci ; free = (slot, WP)
            xb = [big.tile([128, SLOTS, WP], F16, name=f"xb{i}", tag=f"x{i}")
                  for i in range(2)]
            # dense conv out: partition p = 32j + 4k + m ; free=(b, s, g, i*w)
            y_raw = big.tile([128, NB, NS, 4, 448], F32)
            s_sb = small.tile([128, 9, 7, 32], F16)
            stats_buf = small.tile([128, 56, 6], F32)
            sel1_sb = small.tile([128, CO_PC], F32)
            sel2_sb = small.tile([CO_PC, 128], F32)
            g_sb = small.tile([CO_PC, 1], F32)
            b_sb = small.tile([CO_PC, 1], F32)
            stats_sq = small.tile([128, 2], F32)
            msq_scr = small.tile([128, 112], F32)
            red = small.tile([128, 4], F32)
            t4 = small.tile([CO_PC, 2], F32)
            fin = small.tile([CO_PC, 8], F32)
            sb4 = small.tile([CO_PC, 2], F32)
            ab128 = small.tile([128, 2], F32)

            psum_t = psum.tile([128, 8, 512], F32)

            nc.vector.memset(stats_buf[:], 0.0)
            nc.sync.dma_start(out=s_sb[:], in_=s_ext[:])
            nc.sync.dma_start(out=sel1_sb[:], in_=sel1_ext[:])
            nc.sync.dma_start(out=sel2_sb[:], in_=sel2_ext[:])
            nc.sync.dma_start(out=g_sb[:], in_=g_ext[:])
            nc.sync.dma_start(out=b_sb[:], in_=b_ext[:])

            pfull = psum_t[:]
            pstride = pfull.ap[0][0]

            import contextlib
            loop_cm = (tc.For_i(0, loop_n, 1) if loop_n > 1
                       else contextlib.nullcontext())
            with loop_cm:
                _body(nc, tc, locals())
    nc.compile()
    return nc


def _body(nc, tc, env):
    xb = env["xb"]
    y_raw, s_sb = env["y_raw"], env["s_sb"]
    stats_buf, sel1_sb, sel2_sb = (env["stats_buf"], env["sel1_sb"],
                                   env["sel2_sb"])
    g_sb, b_sb = env["g_sb"], env["b_sb"]
    stats_sq, msq_scr, red = env["stats_sq"], env["msq_scr"], env["red"]
    t4, fin, sb4, ab128 = env["t4"], env["fin"], env["sb4"], env["ab128"]
    psum_t, ph2 = env["psum_t"], env["ph2"]
    y_ext, xs_ext = env["y_ext"], env["xs_ext"]
    pfull, pstride = env["pfull"], env["pstride"]
    skip = env["skip"]

    xap = xs_ext.ap()
    yap = y_ext.ap()

    # ---- phase 1: conv per (batch, super) ----
    PP = (H + 2) * WP  # padded image size
    for b in range(NB):
        for s in range(NS):
            idx = b * NS + s
            x_c = xb[idx % 2]
            # padded input rows 56s .. 56s+58 (= image rows 56s-1 .. 56s+57)
            if "xdma" not in skip:
                src = bass.AP(
                    tensor=xap.tensor,
                    offset=xap.offset + 4 * b * C * PP + 56 * s * WP,
                    ap=[[C * PP, 4], [PP, C], [1, SLOTS * WP]])
                nc.sync.dma_start(
                    out=x_c.rearrange("p r w -> p (r w)"), in_=src)
            xv = x_c.rearrange("p r w -> p (r w)")
            bank0 = 4 * (idx % 2)
            for k in range(7 if "mm" not in skip else 0):
                for t in range(9):
                    kh, kw = divmod(t, 3)
                    for j in range(4):
                        off = (2 * (7 * j + k) + kh) * WP + kw
                        for g in range(4):
                            nc.tensor.matmul(
                                psum_t[32 * j:32 * j + 32, bank0 + g, 0:450],
                                s_sb[32 * g:32 * g + 32, t, k, :],
                                xv[32 * g:32 * g + 32, off:off + 450],
                                start=(k == 0 and t == 0),
                                stop=(k == 6 and t == 8),
                                tile_position=(32 * g, 32 * j))
            if "epi" in skip:
                continue
            for g in range(4):
                src = bass.AP(
                    tensor=pfull.tensor,
                    offset=pfull.offset + (bank0 + g) * 512,
                    ap=[[pstride, 124], [226, 2], [1, 224]])
                nc.scalar.copy(y_raw[0:124, b, s, g, :], src)

    if env.get("dbg_ext") is not None:
        nc.sync.dma_start(out=env["dbg_ext"].ap(), in_=y_raw[:])

    if "stats" in skip:
        return

    # ---- bulk bn_stats over the dense accumulator (56 x 512-el chunks) ----
    yflat = y_raw.rearrange("p a b c w -> p (a b c w)")
    for i in range(56):
        nc.vector.bn_stats(out=stats_buf[0:124, i, :],
                           in_=yflat[0:124, 512 * i:512 * i + 512])

    # ---- combine bn_stats chunks -> per-partition (sum, sumsq) [128,2] ----
    stats_fl = stats_buf.rearrange("p s (e t) -> p (s e) t", e=2, t=3)
    means = stats_fl[:, :, 1]
    ctv = stats_fl[:, :, 2]
    nc.vector.tensor_reduce(red[:, 0:1], means, mybir.AxisListType.X,
                            mybir.AluOpType.add)
    nc.vector.tensor_tensor(msq_scr[:], means, means, mybir.AluOpType.mult)
    nc.vector.tensor_reduce(red[:, 1:2], msq_scr[:], mybir.AxisListType.X,
                            mybir.AluOpType.add)
    nc.vector.tensor_reduce(red[:, 2:3], ctv, mybir.AxisListType.X,
                            mybir.AluOpType.add)
    nc.vector.tensor_scalar_mul(stats_sq[:, 0:1], red[:, 0:1], 256.0)
    nc.vector.tensor_scalar_mul(red[:, 3:4], red[:, 1:2], 256.0)
    nc.vector.tensor_tensor(stats_sq[:, 1:2], red[:, 3:4], red[:, 2:3],
                            mybir.AluOpType.add)

    # ---- combine (j,k) lanes: [128,2] -> [4,2] via PE ----
    nc.tensor.matmul(psum_t[0:CO_PC, 0, 0:2], sel1_sb[:], stats_sq[:],
                     start=True, stop=True)
    nc.scalar.copy(t4[:], psum_t[0:CO_PC, 0, 0:2])

    # ---- finalize per-channel scale/shift on partitions 0..3 ----
    mean = fin[:, 0:1]
    msqm = fin[:, 1:2]
    v = fin[:, 2:3]
    rec = fin[:, 3:4]
    a_ = fin[:, 4:5]
    bq = fin[:, 5:6]
    cq = fin[:, 6:7]
    sc = fin[:, 7:8]
    inv_n = float(np.float32(1.0) / np.float32(NTOT))
    nc.vector.tensor_scalar_mul(mean, t4[:, 0:1], inv_n)
    nc.vector.tensor_scalar_mul(msqm, t4[:, 1:2], inv_n)
    nc.vector.tensor_tensor(v, mean, mean, mybir.AluOpType.mult)
    nc.vector.tensor_tensor(v, msqm, v, mybir.AluOpType.subtract)
    nc.vector.tensor_scalar_add(v, v, EPS)
    nc.scalar.activation(rec, v, mybir.ActivationFunctionType.Sqrt)
    nc.vector.reciprocal(rec, rec)
    for _ in range(2):  # Newton polish: rec *= 1.5 - 0.5*v*rec^2
        nc.vector.tensor_tensor(a_, rec, rec, mybir.AluOpType.mult)
        nc.vector.tensor_tensor(bq, v, a_, mybir.AluOpType.mult)
        nc.vector.tensor_scalar(cq, bq, -0.5, 1.5, mybir.AluOpType.mult,
                                mybir.AluOpType.add)
        nc.vector.tensor_tensor(rec, rec, cq, mybir.AluOpType.mult)
    nc.vector.tensor_tensor(sc, g_sb[:], rec, mybir.AluOpType.mult)
    nc.vector.tensor_scalar_mul(sb4[:, 0:1], sc, 2.0)
    nc.vector.tensor_tensor(a_, mean, sc, mybir.AluOpType.mult)
    nc.vector.tensor_tensor(bq, b_sb[:], a_, mybir.AluOpType.subtract)
    nc.vector.tensor_scalar_mul(sb4[:, 1:2], bq, 2.0)

    # broadcast [4,2] -> [128,2]
    nc.tensor.matmul(psum_t[:, 1, 0:2], sel2_sb[:], sb4[:],
                     start=True, stop=True)
    nc.scalar.copy(ab128[:], psum_t[:, 1, 0:2])

    if env.get("dbg2_ext") is not None:
        dscr = env["small"].tile([128, 8], F32, name="dscr")
        nc.vector.memset(dscr[:], 0.0)
        nc.scalar.copy(dscr[:, 0:2], stats_sq[:])
        nc.scalar.copy(dscr[:, 2:4], ab128[:])
        nc.scalar.copy(dscr[0:CO_PC, 4:6], t4[:])
        nc.scalar.copy(dscr[0:CO_PC, 6:8], sb4[:])
        nc.sync.dma_start(out=env["dbg2_ext"].ap(), in_=dscr[:])

    # ---- phase 2: normalize + quantize + writeback, per (b, s) chunk ----
    for b in range(NB):
        for s in range(NS):
            if "ph2" in skip:
                break
            zin = y_raw[0:124, b, s].rearrange("p g w -> p (g w)")
            u = ph2.tile([128, 4 * 448], F32, tag="u")
            nc.scalar.activation(u[0:124], zin,
                                 mybir.ActivationFunctionType.Identity,
                                 bias=ab128[0:124, 1:2],
                                 scale=ab128[0:124, 0:1])
            u2 = ph2.tile([128, 4 * 448], F32, tag="u2")
            nc.vector.tensor_scalar(u2[0:124], u[0:124], MAGIC, MAGIC + 2.0,
                                    mybir.AluOpType.add,
                                    mybir.AluOpType.min)
            o = ph2.tile([128, 4 * 448], BF16, tag="o")
            nc.vector.tensor_scalar(o[0:124], u2[0:124], MAGIC - 2.0, MAGIC,
                                    mybir.AluOpType.max,
                                    mybir.AluOpType.subtract)
            ov = o
            for g in range(4):
                for j in range(4):
                    dst = bass.AP(
                        tensor=yap.tensor,
                        offset=(yap.offset + (4 * b + g) * CO_PC * HWs
                                + (56 * s + 14 * j) * W),
                        ap=[[2 * W, 7], [HWs, CO_PC], [1, 448]])
                    nc.sync.dma_start(
                        out=dst,
                        in_=ov[32 * j:32 * j + 28, g * 448:(g + 1) * 448])


def _get_nc(**kw):
    kw.pop("collective", None)  # compat with old test harness
    key = tuple(sorted((k, tuple(v) if isinstance(v, (list, tuple, set)) else v)
                       for k, v in kw.items()))
    if key not in _cache:
        _cache[key] = _build_nc(**kw)
    return _cache[key]


def _host_consts(weight):
    w_bin = np.where(np.asarray(weight, dtype=np.float32) >= 0, 1.0,
                     -1.0).astype(np.float32)
    # sel over partitions p = 32j + col, col = 4k + m valid when col < 28
    p = np.arange(128)
    col = p % 32
    valid = col < 28
    m_of_p = col % 4
    sel1 = ((m_of_p[:, None] == np.arange(CO_PC)[None, :]) & valid[:, None]
            ).astype(np.float32)
    sel2 = np.ascontiguousarray(sel1.T)
    return w_bin, sel1, sel2


def _stage_s(w_bin, c):
    # S[32g+ci, t, k, 4k+m] = w_bin[4c+m, ci, t]
    wt = np.transpose(w_bin[4 * c:4 * c + 4].reshape(CO_PC, C, 9),
                      (1, 2, 0))  # [ci, t, m]
    s32 = np.zeros((C, 9, 7, 32), dtype=ml_dtypes.float16
                   if hasattr(ml_dtypes, "float16") else np.float16)
    for k in range(7):
        s32[:, :, k, 4 * k:4 * k + 4] = wt
    return np.tile(s32, (4, 1, 1, 1))  # [128, 9, 7, 32]


def make_in_maps(x, weight, gamma, beta):
    xq = np.asarray(x, dtype=np.float32).astype(np.float16)
    xs = np.zeros((16, C, H + 2, WP), dtype=np.float16)
    xs[:, :, 1:225, 1:225] = xq
    w_bin, sel1, sel2 = _host_consts(weight)
    gam = np.asarray(gamma, dtype=np.float32)
    bet = np.asarray(beta, dtype=np.float32)
    in_maps = []
    for c in range(N_CORES):
        in_maps.append({
            "xs": xs, "s": _stage_s(w_bin, c), "sel1": sel1, "sel2": sel2,
            "g": gam[4 * c:4 * c + 4].reshape(CO_PC, 1),
            "b": bet[4 * c:4 * c + 4].reshape(CO_PC, 1)})
    return in_maps


def kernel(x, weight, gamma, beta):
    nc = _get_nc()
    in_maps = make_in_maps(x, weight, gamma, beta)
    res = run_bass_kernel_spmd(nc, in_maps, list(range(N_CORES)))
    out = np.concatenate([res.results[c]["y"] for c in range(N_CORES)], axis=1)
    return out.astype(np.float32) * 0.5



# revision 6
# speedup vs baseline: 2.6544x; 2.6544x over previous
"""Trainium2 Bass kernel for BinarizeConv2d block:
   y = round(2*clip(BN(conv3x3(x, sign(w))), -1, 1))/2

Data-parallel sharding: each of 8 cores convolves 2 images over ALL 32
output channels.  BN needs full-batch stats, and NRT collectives cannot sit
inside a For_i hardware loop in this environment, so the kernel runs as TWO
dispatches:
  A ("stats"): conv -> per-core (sum, sumsq) per channel [32,2]; the host
     adds the 8 cores' partials and derives the BN scale/bias (tiny math).
  B ("final"): conv again (cheaper than a fp32 y round-trip through HBM),
     then fused affine+round(magic)+clip -> bf16 {0..4}; host maps to
     {-1,-.5,0,.5,1}.

Conv lowering (the perf core of this kernel): with 32 output channels per
core we can use M=128 PE columns = 32co x 4 row-phases.  rhs partitions
hold K=96 = 32ci x 3 column-shifted copies of x (kw baked into partition
groups), and 6 matmuls with row-offset rhs (a = -1..4) accumulate a
[128, 224] psum tile covering FOUR output rows: column (32d+co) gets
weights W[co,ci,a+1-d,kw] (zero when a+1-d is outside 0..2).  That is
6 matmuls x 224 cycles per 4 rows versus the old channel-sharded scheme's
~63 tiny matmuls per 2 rows -- ~8x fewer PE cycles and ~24x fewer
instructions (the old kernel was instruction-issue-bound).
"""
import sys
sys.path.insert(0, "/opt/trn_rl_repo")
import numpy as np
import ml_dtypes
import concourse.bass as bass
import concourse.bacc as bacc
import concourse.tile as tile
from concourse import mybir
from concourse.bass_utils import run_bass_kernel_spmd

F32 = mybir.dt.float32
F16 = mybir.dt.float16
BF16 = mybir.dt.bfloat16

N_CORES = 8
IMG_PC = 2        # images per core
C = 32
H = W = 224
WP = 226          # padded width (1 left + 1 right)
HP = 226          # padded height
NSLAB = 4         # row-slabs per image (56 rows each)
SR = 56
SLAB_ROWS = 58    # input rows per slab (56 + 2 halo)
SLAB_ELEMS = SLAB_ROWS * WP  # 13108
GP_SLAB = 14      # 4-row groups per slab
N_GROUPS = IMG_PC * NSLAB * GP_SLAB  # 112
MAGIC = 12582912.0  # 1.5 * 2**23 fp32 round-to-nearest-even trick
EPS = 1e-5
NTOT = float(16 * H * W)  # batch elements per channel (global)
HWs = H * W

_cache = {}
_last_ab = None     # filled by kernel(); test.py reuses it for timing B


def _build_nc(mode="final", loop_n=1):
    assert mode in ("stats", "final")
    nc = bacc.Bacc("TRN2", target_bir_lowering=False, debug=False,
                   num_devices=N_CORES)
    xs_ext = nc.declare_dram_parameter("xs", [IMG_PC, 3, C, HP, WP], F16,
                                       isOutput=False)
    sw_ext = nc.declare_dram_parameter("sw", [96, 6, 128], F16, isOutput=False)
    if mode == "stats":
        sel1_ext = nc.declare_dram_parameter("sel1", [128, C], F32,
                                             isOutput=False)
        st_ext = nc.declare_dram_parameter("st", [C, 2], F32, isOutput=True)
        ab_ext = y_ext = None
    else:
        ab_ext = nc.declare_dram_parameter("ab", [128, 2], F32, isOutput=False)
        y_ext = nc.declare_dram_parameter("y", [IMG_PC, C, H, W], BF16,
                                          isOutput=True)
        sel1_ext = st_ext = None

    with tile.TileContext(nc) as tc:
        with (
            tc.tile_pool(name="big", bufs=1) as big,
            tc.tile_pool(name="small", bufs=1) as small,
            tc.tile_pool(name="ph2", bufs=2) as ph2,
            tc.tile_pool(name="psum", bufs=1, space="PSUM") as psum,
        ):
            xb = [big.tile([96, SLAB_ELEMS], F16, name=f"xb{i}")
                  for i in range(2)]
            # y quarter-buffers: 28 groups each (half an image) so phase 2
            # can stream behind the conv at quarter granularity.
            yq = [big.tile([128, 28, 224], F32, name=f"yq{i}")
                  for i in range(4)]
            s_sb = small.tile([96, 6, 128], F16)
            stats_buf = small.tile([128, 56, 6], F32)
            psum_t = psum.tile([128, 8, 512], F32)

            if mode == "stats":
                sel1_sb = small.tile([128, C], F32)
                st_sb = small.tile([C, 2], F32)
                msq_scr = small.tile([128, 112], F32)
                red = small.tile([128, 4], F32)
                stats_sq = small.tile([128, 2], F32)
                nc.sync.dma_start(out=sel1_sb[:], in_=sel1_ext.ap())
            else:
                ab_sb = small.tile([128, 2], F32)
                nc.sync.dma_start(out=ab_sb[:], in_=ab_ext.ap())
            nc.sync.dma_start(out=s_sb[:], in_=sw_ext.ap())

            env = dict(locals())
            import contextlib
            loop_cm = (tc.For_i(0, loop_n, 1) if loop_n > 1
                       else contextlib.nullcontext())
            with loop_cm:
                _body(nc, tc, env, mode)
    nc.compile()
    return nc


def _body(nc, tc, env, mode):
    xb, yq, s_sb = env["xb"], env["yq"], env["s_sb"]
    stats_buf, psum_t = env["stats_buf"], env["psum_t"]
    xs_ap = env["xs_ext"].ap()

    # ---- conv + drain (+ bn_stats in stats mode) ----
    for slab in range(IMG_PC * NSLAB):
        img, s = divmod(slab, NSLAB)
        x_c = xb[slab % 2]
        src = bass.AP(
            tensor=xs_ap.tensor,
            offset=(xs_ap.offset + img * 3 * C * HP * WP
                    + SR * s * WP),
            ap=[[C * HP * WP, 3], [HP * WP, C], [1, SLAB_ELEMS]])
        nc.sync.dma_start(out=x_c[:], in_=src)
        for j in range(GP_SLAB):
            g_glob = slab * GP_SLAB + j
            bank = g_glob % 8
            hl = 4 * j
            for ai in range(6):
                a = ai - 1
                off = (hl + a + 1) * WP
                nc.tensor.matmul(
                    psum_t[0:128, bank, 0:224],
                    s_sb[0:96, ai, :],
                    x_c[0:96, off:off + 224],
                    start=(ai == 0), stop=(ai == 5))
            if g_glob % 2 == 1:
                # drain the (even, odd) bank pair in one ACT copy
                q, qg = divmod(g_glob - 1, 28)
                pair_src = psum_t[0:128, bank - 1:bank + 1, 0:224]
                nc.scalar.copy(yq[q][0:128, qg:qg + 2, :], pair_src)
                if mode == "stats":
                    pair = (g_glob - 1) // 2
                    nc.vector.bn_stats(
                        out=stats_buf[0:128, pair, :],
                        in_=yq[q][0:128, qg:qg + 2, :].rearrange(
                            "p a b -> p (a b)"))
        if mode == "final":
            # phase 2 on each completed quarter (2 chunks of 14 groups)
            if slab % 2 == 1:
                for ci in range(2):
                    q = slab // 2
                    _phase2_chunk(nc, env, q, ci)

    if mode == "stats":
        _stats_reduce(nc, env)


def _phase2_chunk(nc, env, q, ci):
    """Affine+round+clip 14 groups (a quarter-image half) and DMA out."""
    yq, ab_sb = env["yq"], env["ab_sb"]
    ph2 = env["ph2"]
    y_ap = env["y_ext"].ap()
    img, qi = divmod(q, 2)          # quarter q = image img, half qi
    ng = 14
    zin = yq[q][0:128, ng * ci:ng * (ci + 1), :].rearrange("p a b -> p (a b)")
    n = ng * 224
    u = ph2.tile([128, n], F32, tag="u")
    nc.scalar.activation(u[:], zin,
                         mybir.ActivationFunctionType.Identity,
                         bias=ab_sb[0:128, 1:2],
                         scale=ab_sb[0:128, 0:1])
    nc.vector.tensor_scalar(u[:], u[:], MAGIC, MAGIC + 4.0,
                            mybir.AluOpType.add, mybir.AluOpType.min)
    o = ph2.tile([128, n], BF16, tag="o")
    nc.vector.tensor_scalar(o[:], u[:], MAGIC, MAGIC,
                            mybir.AluOpType.max, mybir.AluOpType.subtract)
    # groups here are rows 4g+d, g in [28*qi+14*ci, +14), d = p//32
    g0 = 28 * qi + ng * ci
    for d in range(4):
        dst = bass.AP(
            tensor=y_ap.tensor,
            offset=y_ap.offset + img * C * HWs + (4 * g0 + d) * W,
            ap=[[HWs, C], [4 * W, ng], [1, W]])
        nc.sync.dma_start(out=dst, in_=o[32 * d:32 * d + 32, :])


def _stats_reduce(nc, env):
    """stats_buf [128,56,6] -> per-channel (sum, sumsq) [32,2] -> DRAM."""
    stats_buf, psum_t = env["stats_buf"], env["psum_t"]
    msq_scr, red, stats_sq = env["msq_scr"], env["red"], env["stats_sq"]
    sel1_sb, st_sb = env["sel1_sb"], env["st_sb"]
    st_ap = env["st_ext"].ap()

    stats_fl = stats_buf.rearrange("p s (e t) -> p (s e) t", e=2, t=3)
    means = stats_fl[:, :, 1]
    ctv = stats_fl[:, :, 2]
    nc.vector.tensor_reduce(red[:, 0:1], means, mybir.AxisListType.X,
                            mybir.AluOpType.add)
    nc.vector.tensor_tensor(msq_scr[:], means, means, mybir.AluOpType.mult)
    nc.vector.tensor_reduce(red[:, 1:2], msq_scr[:], mybir.AxisListType.X,
                            mybir.AluOpType.add)
    nc.vector.tensor_reduce(red[:, 2:3], ctv, mybir.AxisListType.X,
                            mybir.AluOpType.add)
    nc.vector.tensor_scalar_mul(stats_sq[:, 0:1], red[:, 0:1], 224.0)
    nc.vector.tensor_scalar_mul(red[:, 3:4], red[:, 1:2], 224.0)
    nc.vector.tensor_tensor(stats_sq[:, 1:2], red[:, 3:4], red[:, 2:3],
                            mybir.AluOpType.add)
    nc.tensor.matmul(psum_t[0:C, 0, 0:2], sel1_sb[:], stats_sq[:],
                     start=True, stop=True)
    nc.scalar.copy(st_sb[:], psum_t[0:C, 0, 0:2])
    nc.sync.dma_start(out=st_ap, in_=st_sb[:])


def _get_nc(**kw):
    kw.pop("collective", None)
    kw.setdefault("mode", "final")
    key = tuple(sorted(kw.items()))
    if key not in _cache:
        _cache[key] = _build_nc(**kw)
    return _cache[key]


def _prep_x(x):
    """[16,32,224,224] f32 -> per-core [2,3,32,226,226] f16 shifted copies."""
    xq = np.asarray(x, dtype=np.float32).astype(ml_dtypes.float16
                    if hasattr(ml_dtypes, "float16") else np.float16)
    xp = np.zeros((16, C, HP, WP), dtype=xq.dtype)
    xp[:, :, 1:225, 1:225] = xq
    xs3 = np.zeros((16, 3, C, HP, WP), dtype=xq.dtype)
    xs3[:, 0] = xp
    xs3[:, 1, :, :, :WP - 1] = xp[:, :, :, 1:]
    xs3[:, 2, :, :, :WP - 2] = xp[:, :, :, 2:]
    return xs3


def _prep_w(weight):
    """OIHW weight -> lhsT stack sw[96, 6, 128] f16 (binarized)."""
    w_bin = np.where(np.asarray(weight, dtype=np.float32) >= 0, 1.0,
                     -1.0).astype(np.float32)
    sw = np.zeros((96, 6, 128), dtype=np.float32)
    for ai in range(6):
        a = ai - 1
        for d in range(4):
            kh = a + 1 - d
            if 0 <= kh <= 2:
                for kw in range(3):
                    # lhsT[32*kw+ci, ai, 32*d+co] = w_bin[co, ci, kh, kw]
                    sw[32 * kw:32 * kw + 32, ai, 32 * d:32 * d + 32] = \
                        w_bin[:, :, kh, kw].T
    return sw.astype(ml_dtypes.float16
                     if hasattr(ml_dtypes, "float16") else np.float16)


def _sel1():
    p = np.arange(128)
    return (p[:, None] % 32 == np.arange(C)[None, :]).astype(np.float32)


def make_in_maps_A(x, weight):
    xs3 = _prep_x(x)
    sw = _prep_w(weight)
    sel1 = _sel1()
    return [{"xs": xs3[IMG_PC * c:IMG_PC * (c + 1)], "sw": sw, "sel1": sel1}
            for c in range(N_CORES)]


def reduce_stats_host(st_list, gamma, beta):
    """8x [32,2] partials -> ab [128,2] = (2*scale, 2*bias+2) replicated."""
    st = np.sum(np.stack([np.asarray(s, np.float64) for s in st_list]), axis=0)
    mean = st[:, 0] / NTOT
    var = st[:, 1] / NTOT - mean * mean
    rsq = 1.0 / np.sqrt(var + EPS)
    g = np.asarray(gamma, np.float64)
    b = np.asarray(beta, np.float64)
    scale = g * rsq
    bias = b - mean * scale
    ab32 = np.stack([2.0 * scale, 2.0 * bias + 2.0], axis=1).astype(np.float32)
    return np.tile(ab32, (4, 1))    # [128, 2], p = 32d + co


def make_in_maps_B(x, weight, ab):
    xs3 = _prep_x(x)
    sw = _prep_w(weight)
    return [{"xs": xs3[IMG_PC * c:IMG_PC * (c + 1)], "sw": sw, "ab": ab}
            for c in range(N_CORES)]


def kernel(x, weight, gamma, beta):
    global _last_ab
    xs3 = _prep_x(x)
    sw = _prep_w(weight)
    sel1 = _sel1()

    nc_a = _get_nc(mode="stats")
    in_a = [{"xs": xs3[IMG_PC * c:IMG_PC * (c + 1)], "sw": sw, "sel1": sel1}
            for c in range(N_CORES)]
    res_a = run_bass_kernel_spmd(nc_a, in_a, list(range(N_CORES)))
    ab = reduce_stats_host([res_a.results[c]["st"] for c in range(N_CORES)],
                           gamma, beta)
    _last_ab = ab

    nc_b = _get_nc(mode="final")
    in_b = [{"xs": xs3[IMG_PC * c:IMG_PC * (c + 1)], "sw": sw, "ab": ab}
            for c in range(N_CORES)]
    res_b = run_bass_kernel_spmd(nc_b, in_b, list(range(N_CORES)))
    out = np.concatenate([res_b.results[c]["y"] for c in range(N_CORES)],
                         axis=0)
    return (out.astype(np.float32) - 2.0) * 0.5


# revision 15
# speedup vs baseline: 3.3914x; 1.2777x over previous
"""Trainium2 Bass kernel for BinarizeConv2d block:
   y = round(2*clip(BN(conv3x3(x, sign(w))), -1, 1))/2

Data-parallel sharding: each of 8 cores convolves 2 images over ALL 32
output channels.  BN needs full-batch stats, and NRT collectives cannot sit
inside a For_i hardware loop in this environment, so the kernel runs as TWO
dispatches:
  A ("stats"): conv -> per-core (sum, sumsq) per channel [32,2]; the host
     adds the 8 cores' partials and derives the BN scale/bias (tiny math).
  B ("final"): conv again (cheaper than a fp32 y round-trip through HBM),
     then fused affine+round(magic)+clip -> bf16 {0..4}; host maps to
     {-1,-.5,0,.5,1}.

Conv lowering (the perf core of this kernel): with 32 output channels per
core we can use M=128 PE columns = 32co x 4 row-phases.  rhs partitions
hold K=96 = 32ci x 3 column-shifted copies of x (kw baked into partition
groups), and 6 matmuls with row-offset rhs (a = -1..4) accumulate a
[128, 224] psum tile covering FOUR output rows: column (32d+co) gets
weights W[co,ci,a+1-d,kw] (zero when a+1-d is outside 0..2).  That is
6 matmuls x 224 cycles per 4 rows versus the old channel-sharded scheme's
~63 tiny matmuls per 2 rows -- ~8x fewer PE cycles and ~24x fewer
instructions (the old kernel was instruction-issue-bound).
"""
import sys
sys.path.insert(0, "/opt/trn_rl_repo")
import numpy as np
import ml_dtypes
import concourse.bass as bass
import concourse.bacc as bacc
import concourse.tile as tile
from concourse import mybir
from concourse.bass_utils import run_bass_kernel_spmd

F32 = mybir.dt.float32
F16 = mybir.dt.float16
BF16 = mybir.dt.bfloat16

N_CORES = 8
IMG_PC = 2        # images per core
C = 32
H = W = 224
WP = 226          # padded width (1 left + 1 right)
HP = 226          # padded height
NSLAB = 4         # row-slabs per image (56 rows each)
SR = 56
SLAB_ROWS = 58    # input rows per slab (56 + 2 halo)
SLAB_ELEMS = SLAB_ROWS * WP  # 13108
GP_SLAB = 14      # 4-row groups per slab
N_GROUPS = IMG_PC * NSLAB * GP_SLAB  # 112
MAGIC = 12582912.0  # 1.5 * 2**23 fp32 round-to-nearest-even trick
EPS = 1e-5
NTOT = float(16 * H * W)  # batch elements per channel (global)
HWs = H * W

_cache = {}
_last_ab = None     # filled by kernel(); test.py reuses them for timing B
_last_yr = None


def _build_nc(mode="final", loop_n=1, skip=()):
    assert mode in ("stats", "final")
    nc = bacc.Bacc("TRN2", target_bir_lowering=False, debug=False,
                   num_devices=N_CORES)
    if mode == "stats":
        xs_ext = nc.declare_dram_parameter("xs", [IMG_PC, 3, C, HP, WP], F16,
                                           isOutput=False)
        sw_ext = nc.declare_dram_parameter("sw", [96, 6, 128], F16,
                                           isOutput=False)
        sel1_ext = nc.declare_dram_parameter("sel1", [128, C], F32,
                                             isOutput=False)
        st_ext = nc.declare_dram_parameter("st", [C, 2], F32, isOutput=True)
        yr_ext = nc.declare_dram_parameter("yr", [4, 128, 28, 224], F32,
                                           isOutput=True)
        ab_ext = y_ext = None
    else:
        yr_ext = nc.declare_dram_parameter("yr", [4, 128, 28, 224], F32,
                                           isOutput=False)
        ab_ext = nc.declare_dram_parameter("ab", [128, 2], F32, isOutput=False)
        y_ext = nc.declare_dram_parameter("y", [IMG_PC, C, H, W], BF16,
                                          isOutput=True)
        xs_ext = sw_ext = sel1_ext = st_ext = None

    with tile.TileContext(nc) as tc:
        with (
            tc.tile_pool(name="big", bufs=1) as big,
            tc.tile_pool(name="small", bufs=1) as small,
            tc.tile_pool(name="ph2", bufs=2) as ph2,
            tc.tile_pool(name="psum", bufs=1, space="PSUM") as psum,
        ):
            # y quarter-buffers: 28 groups each (half an image) so the
            # epilogue / writeback can stream at quarter granularity.
            yq = [big.tile([128, 28, 224], F32, name=f"yq{i}")
                  for i in range(4)]
            psum_t = psum.tile([128, 8, 512], F32)
            if mode == "stats":
                xb = [big.tile([96, SLAB_ELEMS], F16, name=f"xb{i}")
                      for i in range(2)]
                s_sb = small.tile([96, 6, 128], F16)
                stats_buf = small.tile([128, 56, 6], F32)
                sel1_sb = small.tile([128, C], F32)
                st_sb = small.tile([C, 2], F32)
                msq_scr = small.tile([128, 112], F32)
                red = small.tile([128, 4], F32)
                stats_sq = small.tile([128, 2], F32)
                nc.sync.dma_start(out=sel1_sb[:], in_=sel1_ext.ap())
                nc.sync.dma_start(out=s_sb[:], in_=sw_ext.ap())
            else:
                ab_sb = small.tile([128, 2], F32)
                nc.sync.dma_start(out=ab_sb[:], in_=ab_ext.ap())

            env = dict(locals())
            import contextlib
            loop_cm = (tc.For_i(0, loop_n, 1) if loop_n > 1
                       else contextlib.nullcontext())
            with loop_cm:
                if mode == "stats":
                    _body_stats(nc, tc, env, skip)
                else:
                    _body_final(nc, tc, env, skip)
    nc.compile()
    return nc


def _body_stats(nc, tc, env, skip=()):
    """Dispatch A: conv -> yq quarters -> yr HBM dump + bn stats -> st."""
    xb, yq, s_sb = env["xb"], env["yq"], env["s_sb"]
    stats_buf, psum_t = env["stats_buf"], env["psum_t"]
    xs_ap = env["xs_ext"].ap()
    yr_ap = env["yr_ext"].ap()

    for slab in range(IMG_PC * NSLAB):
        img, s = divmod(slab, NSLAB)
        x_c = xb[slab % 2]
        if "xdma" not in skip:
            src = bass.AP(
                tensor=xs_ap.tensor,
                offset=(xs_ap.offset + img * 3 * C * HP * WP
                        + SR * s * WP),
                ap=[[C * HP * WP, 3], [HP * WP, C], [1, SLAB_ELEMS]])
            nc.sync.dma_start(out=x_c[:], in_=src)
        for j in range(GP_SLAB if "mm" not in skip else 0):
            g_glob = slab * GP_SLAB + j
            bank = g_glob % 8
            hl = 4 * j
            for ai in range(6):
                a = ai - 1
                off = (hl + a + 1) * WP
                nc.tensor.matmul(
                    psum_t[0:128, bank, 0:224],
                    s_sb[0:96, ai, :],
                    x_c[0:96, off:off + 224],
                    start=(ai == 0), stop=(ai == 5))
            if g_glob % 2 == 1 and "drain" not in skip:
                # drain the (even, odd) bank pair in one ACT copy
                q, qg = divmod(g_glob - 1, 28)
                pair_src = psum_t[0:128, bank - 1:bank + 1, 0:224]
                nc.scalar.copy(yq[q][0:128, qg:qg + 2, :], pair_src)
                if "stats" not in skip:
                    pair = (g_glob - 1) // 2
                    nc.vector.bn_stats(
                        out=stats_buf[0:128, pair, :],
                        in_=yq[q][0:128, qg:qg + 2, :].rearrange(
                            "p a b -> p (a b)"))
        if slab % 2 == 1 and "ydma" not in skip and "drain" not in skip:
            q = slab // 2
            dst = bass.AP(
                tensor=yr_ap.tensor,
                offset=yr_ap.offset + q * 128 * 28 * 224,
                ap=[[28 * 224, 128], [1, 28 * 224]])
            nc.sync.dma_start(
                out=dst, in_=yq[q][:].rearrange("p a b -> p (a b)"))

    if "stats" not in skip and "drain" not in skip:
        _stats_reduce(nc, env)


def _body_final(nc, tc, env, skip=()):
    """Dispatch B: yr HBM -> yq -> affine+round+clip -> y NCHW bf16."""
    yq = env["yq"]
    yr_ap = env["yr_ext"].ap()
    for q in range(4):
        if "ydma" not in skip:
            src = bass.AP(
                tensor=yr_ap.tensor,
                offset=yr_ap.offset + q * 128 * 28 * 224,
                ap=[[28 * 224, 128], [1, 28 * 224]])
            nc.sync.dma_start(
                out=yq[q][:].rearrange("p a b -> p (a b)"), in_=src)
        if "ph2" not in skip:
            for ci in range(2):
                _phase2_chunk(nc, env, q, ci)


def _phase2_chunk(nc, env, q, ci):
    """Affine+round+clip 14 groups (a quarter-image half) and DMA out."""
    yq, ab_sb = env["yq"], env["ab_sb"]
    ph2 = env["ph2"]
    y_ap = env["y_ext"].ap()
    img, qi = divmod(q, 2)          # quarter q = image img, half qi
    ng = 14
    zin = yq[q][0:128, ng * ci:ng * (ci + 1), :].rearrange("p a b -> p (a b)")
    n = ng * 224
    u = ph2.tile([128, n], F32, tag="u")
    nc.scalar.activation(u[:], zin,
                         mybir.ActivationFunctionType.Identity,
                         bias=ab_sb[0:128, 1:2],
                         scale=ab_sb[0:128, 0:1])
    nc.vector.tensor_scalar(u[:], u[:], MAGIC, MAGIC + 4.0,
                            mybir.AluOpType.add, mybir.AluOpType.min)
    o = ph2.tile([128, n], BF16, tag="o")
    nc.vector.tensor_scalar(o[:], u[:], MAGIC, MAGIC,
                            mybir.AluOpType.max, mybir.AluOpType.subtract)
    # groups here are rows 4g+d, g in [28*qi+14*ci, +14), d = p//32
    g0 = 28 * qi + ng * ci
    for d in range(4):
        dst = bass.AP(
            tensor=y_ap.tensor,
            offset=y_ap.offset + img * C * HWs + (4 * g0 + d) * W,
            ap=[[HWs, C], [4 * W, ng], [1, W]])
        nc.sync.dma_start(out=dst, in_=o[32 * d:32 * d + 32, :])


def _stats_reduce(nc, env):
    """stats_buf [128,56,6] -> per-channel (sum, sumsq) [32,2] -> DRAM."""
    stats_buf, psum_t = env["stats_buf"], env["psum_t"]
    msq_scr, red, stats_sq = env["msq_scr"], env["red"], env["stats_sq"]
    sel1_sb, st_sb = env["sel1_sb"], env["st_sb"]
    st_ap = env["st_ext"].ap()

    stats_fl = stats_buf.rearrange("p s (e t) -> p (s e) t", e=2, t=3)
    means = stats_fl[:, :, 1]
    ctv = stats_fl[:, :, 2]
    nc.vector.tensor_reduce(red[:, 0:1], means, mybir.AxisListType.X,
                            mybir.AluOpType.add)
    nc.vector.tensor_tensor(msq_scr[:], means, means, mybir.AluOpType.mult)
    nc.vector.tensor_reduce(red[:, 1:2], msq_scr[:], mybir.AxisListType.X,
                            mybir.AluOpType.add)
    nc.vector.tensor_reduce(red[:, 2:3], ctv, mybir.AxisListType.X,
                            mybir.AluOpType.add)
    nc.vector.tensor_scalar_mul(stats_sq[:, 0:1], red[:, 0:1], 224.0)
    nc.vector.tensor_scalar_mul(red[:, 3:4], red[:, 1:2], 224.0)
    nc.vector.tensor_tensor(stats_sq[:, 1:2], red[:, 3:4], red[:, 2:3],
                            mybir.AluOpType.add)
    nc.tensor.matmul(psum_t[0:C, 0, 0:2], sel1_sb[:], stats_sq[:],
                     start=True, stop=True)
    nc.scalar.copy(st_sb[:], psum_t[0:C, 0, 0:2])
    nc.sync.dma_start(out=st_ap, in_=st_sb[:])


def _get_nc(**kw):
    kw.pop("collective", None)
    kw.setdefault("mode", "final")
    key = tuple(sorted((k, tuple(v) if isinstance(v, (list, tuple, set)) else v)
                       for k, v in kw.items()))
    if key not in _cache:
        _cache[key] = _build_nc(**kw)
    return _cache[key]


def _prep_x(x):
    """[16,32,224,224] f32 -> per-core [2,3,32,226,226] f16 shifted copies."""
    xq = np.asarray(x, dtype=np.float32).astype(ml_dtypes.float16
                    if hasattr(ml_dtypes, "float16") else np.float16)
    xp = np.zeros((16, C, HP, WP), dtype=xq.dtype)
    xp[:, :, 1:225, 1:225] = xq
    xs3 = np.zeros((16, 3, C, HP, WP), dtype=xq.dtype)
    xs3[:, 0] = xp
    xs3[:, 1, :, :, :WP - 1] = xp[:, :, :, 1:]
    xs3[:, 2, :, :, :WP - 2] = xp[:, :, :, 2:]
    return xs3


def _prep_w(weight):
    """OIHW weight -> lhsT stack sw[96, 6, 128] f16 (binarized)."""
    w_bin = np.where(np.asarray(weight, dtype=np.float32) >= 0, 1.0,
                     -1.0).astype(np.float32)
    sw = np.zeros((96, 6, 128), dtype=np.float32)
    for ai in range(6):
        a = ai - 1
        for d in range(4):
            kh = a + 1 - d
            if 0 <= kh <= 2:
                for kw in range(3):
                    # lhsT[32*kw+ci, ai, 32*d+co] = w_bin[co, ci, kh, kw]
                    sw[32 * kw:32 * kw + 32, ai, 32 * d:32 * d + 32] = \
                        w_bin[:, :, kh, kw].T
    return sw.astype(ml_dtypes.float16
                     if hasattr(ml_dtypes, "float16") else np.float16)


def _sel1():
    p = np.arange(128)
    return (p[:, None] % 32 == np.arange(C)[None, :]).astype(np.float32)


def make_in_maps_A(x, weight):
    xs3 = _prep_x(x)
    sw = _prep_w(weight)
    sel1 = _sel1()
    return [{"xs": xs3[IMG_PC * c:IMG_PC * (c + 1)], "sw": sw, "sel1": sel1}
            for c in range(N_CORES)]


def make_in_maps_B(yr_list, ab):
    return [{"yr": yr_list[c], "ab": ab} for c in range(N_CORES)]


def reduce_stats_host(st_list, gamma, beta):
    """8x [32,2] partials -> ab [128,2] = (2*scale, 2*bias+2) replicated."""
    st = np.sum(np.stack([np.asarray(s, np.float64) for s in st_list]), axis=0)
    mean = st[:, 0] / NTOT
    var = st[:, 1] / NTOT - mean * mean
    rsq = 1.0 / np.sqrt(var + EPS)
    g = np.asarray(gamma, np.float64)
    b = np.asarray(beta, np.float64)
    scale = g * rsq
    bias = b - mean * scale
    ab32 = np.stack([2.0 * scale, 2.0 * bias + 2.0], axis=1).astype(np.float32)
    return np.tile(ab32, (4, 1))    # [128, 2], p = 32d + co


def kernel(x, weight, gamma, beta):
    global _last_ab, _last_yr
    nc_a = _get_nc(mode="stats")
    in_a = make_in_maps_A(x, weight)
    res_a = run_bass_kernel_spmd(nc_a, in_a, list(range(N_CORES)))
    ab = reduce_stats_host([res_a.results[c]["st"] for c in range(N_CORES)],
                           gamma, beta)
    yr_list = [np.asarray(res_a.results[c]["yr"]) for c in range(N_CORES)]
    _last_ab, _last_yr = ab, yr_list

    nc_b = _get_nc(mode="final")
    in_b = make_in_maps_B(yr_list, ab)
    res_b = run_bass_kernel_spmd(nc_b, in_b, list(range(N_CORES)))
    out = np.concatenate([res_b.results[c]["y"] for c in range(N_CORES)],
                         axis=0)
    return (out.astype(np.float32) - 2.0) * 0.5


# revision 16
# speedup vs baseline: 3.7159x; 1.0957x over previous
"""Trainium2 Bass kernel for BinarizeConv2d block:
   y = round(2*clip(BN(conv3x3(x, sign(w))), -1, 1))/2

Data-parallel sharding: each of 8 cores convolves 2 images over ALL 32
output channels.  BN needs full-batch stats, and NRT collectives cannot sit
inside a For_i hardware loop in this environment, so the kernel runs as TWO
dispatches:
  A ("stats"): conv -> per-core (sum, sumsq) per channel [32,2]; the host
     adds the 8 cores' partials and derives the BN scale/bias (tiny math).
  B ("final"): conv again (cheaper than a fp32 y round-trip through HBM),
     then fused affine+round(magic)+clip -> bf16 {0..4}; host maps to
     {-1,-.5,0,.5,1}.

Conv lowering (the perf core of this kernel): with 32 output channels per
core we can use M=128 PE columns = 32co x 4 row-phases.  rhs partitions
hold K=96 = 32ci x 3 column-shifted copies of x (kw baked into partition
groups), and 6 matmuls with row-offset rhs (a = -1..4) accumulate a
[128, 224] psum tile covering FOUR output rows: column (32d+co) gets
weights W[co,ci,a+1-d,kw] (zero when a+1-d is outside 0..2).  That is
6 matmuls x 224 cycles per 4 rows versus the old channel-sharded scheme's
~63 tiny matmuls per 2 rows -- ~8x fewer PE cycles and ~24x fewer
instructions (the old kernel was instruction-issue-bound).
"""
import sys
sys.path.insert(0, "/opt/trn_rl_repo")
import numpy as np
import ml_dtypes
import concourse.bass as bass
import concourse.bacc as bacc
import concourse.tile as tile
from concourse import mybir
from concourse.bass_utils import run_bass_kernel_spmd

F32 = mybir.dt.float32
F16 = mybir.dt.float16
BF16 = mybir.dt.bfloat16

N_CORES = 8
IMG_PC = 2        # images per core
C = 32
H = W = 224
WP = 226          # padded width (1 left + 1 right)
HP = 226          # padded height
NSLAB = 4         # row-slabs per image (56 rows each)
SR = 56
SLAB_ROWS = 58    # input rows per slab (56 + 2 halo)
SLAB_ELEMS = SLAB_ROWS * WP  # 13108
GP_SLAB = 14      # 4-row groups per slab
N_GROUPS = IMG_PC * NSLAB * GP_SLAB  # 112
MAGIC = 12582912.0  # 1.5 * 2**23 fp32 round-to-nearest-even trick
EPS = 1e-5
NTOT = float(16 * H * W)  # batch elements per channel (global)
HWs = H * W

_cache = {}
_last_ab = None     # filled by kernel(); test.py reuses them for timing B
_last_yr = None


def _build_nc(mode="final", loop_n=1, skip=()):
    assert mode in ("stats", "final")
    nc = bacc.Bacc("TRN2", target_bir_lowering=False, debug=False,
                   num_devices=N_CORES)
    if mode == "stats":
        xs_ext = nc.declare_dram_parameter("xs", [IMG_PC, 3, C, HP, WP], F16,
                                           isOutput=False)
        sw_ext = nc.declare_dram_parameter("sw", [96, 6, 128], F16,
                                           isOutput=False)
        sel1_ext = nc.declare_dram_parameter("sel1", [128, C], F32,
                                             isOutput=False)
        st_ext = nc.declare_dram_parameter("st", [C, 2], F32, isOutput=True)
        yr_ext = nc.declare_dram_parameter("yr", [4, 128, 28, 224], F16,
                                           isOutput=True)
        ab_ext = y_ext = None
    else:
        yr_ext = nc.declare_dram_parameter("yr", [4, 128, 28, 224], F16,
                                           isOutput=False)
        ab_ext = nc.declare_dram_parameter("ab", [128, 2], F32, isOutput=False)
        y_ext = nc.declare_dram_parameter("y", [IMG_PC, C, H, W], BF16,
                                          isOutput=True)
        xs_ext = sw_ext = sel1_ext = st_ext = None

    with tile.TileContext(nc) as tc:
        with (
            tc.tile_pool(name="big", bufs=1) as big,
            tc.tile_pool(name="small", bufs=1) as small,
            tc.tile_pool(name="ph2", bufs=2) as ph2,
            tc.tile_pool(name="psum", bufs=1, space="PSUM") as psum,
        ):
            # y quarter-buffers: 28 groups each (half an image) so the
            # epilogue / writeback can stream at quarter granularity.
            yq = [big.tile([128, 28, 224], F16, name=f"yq{i}")
                  for i in range(4)]
            psum_t = psum.tile([128, 8, 512], F32)
            if mode == "stats":
                xb = [big.tile([96, SLAB_ELEMS], F16, name=f"xb{i}")
                      for i in range(2)]
                s_sb = small.tile([96, 6, 128], F16)
                stats_buf = small.tile([128, 56, 6], F32)
                sel1_sb = small.tile([128, C], F32)
                st_sb = small.tile([C, 2], F32)
                msq_scr = small.tile([128, 112], F32)
                red = small.tile([128, 4], F32)
                stats_sq = small.tile([128, 2], F32)
                nc.sync.dma_start(out=sel1_sb[:], in_=sel1_ext.ap())
                nc.sync.dma_start(out=s_sb[:], in_=sw_ext.ap())
            else:
                ab_sb = small.tile([128, 2], F32)
                nc.sync.dma_start(out=ab_sb[:], in_=ab_ext.ap())

            env = dict(locals())
            import contextlib
            loop_cm = (tc.For_i(0, loop_n, 1) if loop_n > 1
                       else contextlib.nullcontext())
            with loop_cm:
                if mode == "stats":
                    _body_stats(nc, tc, env, skip)
                else:
                    _body_final(nc, tc, env, skip)
    nc.compile()
    return nc


def _body_stats(nc, tc, env, skip=()):
    """Dispatch A: conv -> yq quarters -> yr HBM dump + bn stats -> st."""
    xb, yq, s_sb = env["xb"], env["yq"], env["s_sb"]
    stats_buf, psum_t = env["stats_buf"], env["psum_t"]
    xs_ap = env["xs_ext"].ap()
    yr_ap = env["yr_ext"].ap()

    for slab in range(IMG_PC * NSLAB):
        img, s = divmod(slab, NSLAB)
        x_c = xb[slab % 2]
        if "xdma" not in skip:
            src = bass.AP(
                tensor=xs_ap.tensor,
                offset=(xs_ap.offset + img * 3 * C * HP * WP
                        + SR * s * WP),
                ap=[[C * HP * WP, 3], [HP * WP, C], [1, SLAB_ELEMS]])
            nc.sync.dma_start(out=x_c[:], in_=src)
        for j in range(GP_SLAB if "mm" not in skip else 0):
            g_glob = slab * GP_SLAB + j
            bank = g_glob % 8
            hl = 4 * j
            for ai in range(6):
                a = ai - 1
                off = (hl + a + 1) * WP
                nc.tensor.matmul(
                    psum_t[0:128, bank, 0:224],
                    s_sb[0:96, ai, :],
                    x_c[0:96, off:off + 224],
                    start=(ai == 0), stop=(ai == 5))
            if g_glob % 2 == 1 and "drain" not in skip:
                # drain the (even, odd) bank pair in one ACT copy
                q, qg = divmod(g_glob - 1, 28)
                pair_src = psum_t[0:128, bank - 1:bank + 1, 0:224]
                nc.scalar.copy(yq[q][0:128, qg:qg + 2, :], pair_src)
                if "stats" not in skip:
                    pair = (g_glob - 1) // 2
                    nc.vector.bn_stats(
                        out=stats_buf[0:128, pair, :],
                        in_=yq[q][0:128, qg:qg + 2, :].rearrange(
                            "p a b -> p (a b)"))
        if slab % 2 == 1 and "ydma" not in skip and "drain" not in skip:
            q = slab // 2
            dst = bass.AP(
                tensor=yr_ap.tensor,
                offset=yr_ap.offset + q * 128 * 28 * 224,
                ap=[[28 * 224, 128], [1, 28 * 224]])
            nc.sync.dma_start(
                out=dst, in_=yq[q][:].rearrange("p a b -> p (a b)"))

    if "stats" not in skip and "drain" not in skip:
        _stats_reduce(nc, env)


def _body_final(nc, tc, env, skip=()):
    """Dispatch B: yr HBM -> yq -> affine+round+clip -> y NCHW bf16."""
    yq = env["yq"]
    yr_ap = env["yr_ext"].ap()
    for q in range(4):
        if "ydma" not in skip:
            src = bass.AP(
                tensor=yr_ap.tensor,
                offset=yr_ap.offset + q * 128 * 28 * 224,
                ap=[[28 * 224, 128], [1, 28 * 224]])
            nc.sync.dma_start(
                out=yq[q][:].rearrange("p a b -> p (a b)"), in_=src)
        if "ph2" not in skip:
            for ci in range(2):
                _phase2_chunk(nc, env, q, ci)


def _phase2_chunk(nc, env, q, ci):
    """Affine+round+clip 14 groups (a quarter-image half) and DMA out."""
    yq, ab_sb = env["yq"], env["ab_sb"]
    ph2 = env["ph2"]
    y_ap = env["y_ext"].ap()
    img, qi = divmod(q, 2)          # quarter q = image img, half qi
    ng = 14
    zin = yq[q][0:128, ng * ci:ng * (ci + 1), :].rearrange("p a b -> p (a b)")
    n = ng * 224
    u = ph2.tile([128, n], F32, tag="u")
    nc.scalar.activation(u[:], zin,
                         mybir.ActivationFunctionType.Identity,
                         bias=ab_sb[0:128, 1:2],
                         scale=ab_sb[0:128, 0:1])
    nc.vector.tensor_scalar(u[:], u[:], MAGIC, MAGIC + 4.0,
                            mybir.AluOpType.add, mybir.AluOpType.min)
    o = ph2.tile([128, n], BF16, tag="o")
    nc.vector.tensor_scalar(o[:], u[:], MAGIC, MAGIC,
                            mybir.AluOpType.max, mybir.AluOpType.subtract)
    # groups here are rows 4g+d, g in [28*qi+14*ci, +14), d = p//32
    g0 = 28 * qi + ng * ci
    for d in range(4):
        dst = bass.AP(
            tensor=y_ap.tensor,
            offset=y_ap.offset + img * C * HWs + (4 * g0 + d) * W,
            ap=[[HWs, C], [4 * W, ng], [1, W]])
        nc.sync.dma_start(out=dst, in_=o[32 * d:32 * d + 32, :])


def _stats_reduce(nc, env):
    """stats_buf [128,56,6] -> per-channel (sum, sumsq) [32,2] -> DRAM."""
    stats_buf, psum_t = env["stats_buf"], env["psum_t"]
    msq_scr, red, stats_sq = env["msq_scr"], env["red"], env["stats_sq"]
    sel1_sb, st_sb = env["sel1_sb"], env["st_sb"]
    st_ap = env["st_ext"].ap()

    stats_fl = stats_buf.rearrange("p s (e t) -> p (s e) t", e=2, t=3)
    means = stats_fl[:, :, 1]
    ctv = stats_fl[:, :, 2]
    nc.vector.tensor_reduce(red[:, 0:1], means, mybir.AxisListType.X,
                            mybir.AluOpType.add)
    nc.vector.tensor_tensor(msq_scr[:], means, means, mybir.AluOpType.mult)
    nc.vector.tensor_reduce(red[:, 1:2], msq_scr[:], mybir.AxisListType.X,
                            mybir.AluOpType.add)
    nc.vector.tensor_reduce(red[:, 2:3], ctv, mybir.AxisListType.X,
                            mybir.AluOpType.add)
    nc.vector.tensor_scalar_mul(stats_sq[:, 0:1], red[:, 0:1], 224.0)
    nc.vector.tensor_scalar_mul(red[:, 3:4], red[:, 1:2], 224.0)
    nc.vector.tensor_tensor(stats_sq[:, 1:2], red[:, 3:4], red[:, 2:3],
                            mybir.AluOpType.add)
    nc.tensor.matmul(psum_t[0:C, 0, 0:2], sel1_sb[:], stats_sq[:],
                     start=True, stop=True)
    nc.scalar.copy(st_sb[:], psum_t[0:C, 0, 0:2])
    nc.sync.dma_start(out=st_ap, in_=st_sb[:])


def _get_nc(**kw):
    kw.pop("collective", None)
    kw.setdefault("mode", "final")
    key = tuple(sorted((k, tuple(v) if isinstance(v, (list, tuple, set)) else v)
                       for k, v in kw.items()))
    if key not in _cache:
        _cache[key] = _build_nc(**kw)
    return _cache[key]


def _prep_x(x):
    """[16,32,224,224] f32 -> per-core [2,3,32,226,226] f16 shifted copies."""
    xq = np.asarray(x, dtype=np.float32).astype(ml_dtypes.float16
                    if hasattr(ml_dtypes, "float16") else np.float16)
    xp = np.zeros((16, C, HP, WP), dtype=xq.dtype)
    xp[:, :, 1:225, 1:225] = xq
    xs3 = np.zeros((16, 3, C, HP, WP), dtype=xq.dtype)
    xs3[:, 0] = xp
    xs3[:, 1, :, :, :WP - 1] = xp[:, :, :, 1:]
    xs3[:, 2, :, :, :WP - 2] = xp[:, :, :, 2:]
    return xs3


def _prep_w(weight):
    """OIHW weight -> lhsT stack sw[96, 6, 128] f16 (binarized)."""
    w_bin = np.where(np.asarray(weight, dtype=np.float32) >= 0, 1.0,
                     -1.0).astype(np.float32)
    sw = np.zeros((96, 6, 128), dtype=np.float32)
    for ai in range(6):
        a = ai - 1
        for d in range(4):
            kh = a + 1 - d
            if 0 <= kh <= 2:
                for kw in range(3):
                    # lhsT[32*kw+ci, ai, 32*d+co] = w_bin[co, ci, kh, kw]
                    sw[32 * kw:32 * kw + 32, ai, 32 * d:32 * d + 32] = \
                        w_bin[:, :, kh, kw].T
    return sw.astype(ml_dtypes.float16
                     if hasattr(ml_dtypes, "float16") else np.float16)


def _sel1():
    p = np.arange(128)
    return (p[:, None] % 32 == np.arange(C)[None, :]).astype(np.float32)


def make_in_maps_A(x, weight):
    xs3 = _prep_x(x)
    sw = _prep_w(weight)
    sel1 = _sel1()
    return [{"xs": xs3[IMG_PC * c:IMG_PC * (c + 1)], "sw": sw, "sel1": sel1}
            for c in range(N_CORES)]


def make_in_maps_B(yr_list, ab):
    return [{"yr": yr_list[c], "ab": ab} for c in range(N_CORES)]


def reduce_stats_host(st_list, gamma, beta):
    """8x [32,2] partials -> ab [128,2] = (2*scale, 2*bias+2) replicated."""
    st = np.sum(np.stack([np.asarray(s, np.float64) for s in st_list]), axis=0)
    mean = st[:, 0] / NTOT
    var = st[:, 1] / NTOT - mean * mean
    rsq = 1.0 / np.sqrt(var + EPS)
    g = np.asarray(gamma, np.float64)
    b = np.asarray(beta, np.float64)
    scale = g * rsq
    bias = b - mean * scale
    ab32 = np.stack([2.0 * scale, 2.0 * bias + 2.0], axis=1).astype(np.float32)
    return np.tile(ab32, (4, 1))    # [128, 2], p = 32d + co


def kernel(x, weight, gamma, beta):
    global _last_ab, _last_yr
    nc_a = _get_nc(mode="stats")
    in_a = make_in_maps_A(x, weight)
    res_a = run_bass_kernel_spmd(nc_a, in_a, list(range(N_CORES)))
    ab = reduce_stats_host([res_a.results[c]["st"] for c in range(N_CORES)],
                           gamma, beta)
    yr_list = [np.asarray(res_a.results[c]["yr"]) for c in range(N_CORES)]
    _last_ab, _last_yr = ab, yr_list

    nc_b = _get_nc(mode="final")
    in_b = make_in_maps_B(yr_list, ab)
    res_b = run_bass_kernel_spmd(nc_b, in_b, list(range(N_CORES)))
    out = np.concatenate([res_b.results[c]["y"] for c in range(N_CORES)],
                         axis=0)
    return (out.astype(np.float32) - 2.0) * 0.5


# revision 19
# speedup vs baseline: 4.0782x; 1.0975x over previous
"""Trainium2 Bass kernel for BinarizeConv2d block:
   y = round(2*clip(BN(conv3x3(x, sign(w))), -1, 1))/2

Data-parallel sharding: each of 8 cores convolves 2 images over ALL 32
output channels.  BN needs full-batch stats, and NRT collectives cannot sit
inside a For_i hardware loop in this environment, so the kernel runs as TWO
dispatches:
  A ("stats"): conv -> per-core (sum, sumsq) per channel [32,2]; the host
     adds the 8 cores' partials and derives the BN scale/bias (tiny math).
  B ("final"): conv again (cheaper than a fp32 y round-trip through HBM),
     then fused affine+round(magic)+clip -> bf16 {0..4}; host maps to
     {-1,-.5,0,.5,1}.

Conv lowering (the perf core of this kernel): with 32 output channels per
core we can use M=128 PE columns = 32co x 4 row-phases.  rhs partitions
hold K=96 = 32ci x 3 column-shifted copies of x (kw baked into partition
groups), and 6 matmuls with row-offset rhs (a = -1..4) accumulate a
[128, 224] psum tile covering FOUR output rows: column (32d+co) gets
weights W[co,ci,a+1-d,kw] (zero when a+1-d is outside 0..2).  That is
6 matmuls x 224 cycles per 4 rows versus the old channel-sharded scheme's
~63 tiny matmuls per 2 rows -- ~8x fewer PE cycles and ~24x fewer
instructions (the old kernel was instruction-issue-bound).
"""
import sys
sys.path.insert(0, "/opt/trn_rl_repo")
import numpy as np
import ml_dtypes
import concourse.bass as bass
import concourse.bacc as bacc
import concourse.tile as tile
from concourse import mybir
from concourse.bass_utils import run_bass_kernel_spmd

F32 = mybir.dt.float32
F16 = mybir.dt.float16
BF16 = mybir.dt.bfloat16
I8 = mybir.dt.int8

N_CORES = 8
IMG_PC = 2        # images per core
C = 32
H = W = 224
WP = 226          # padded width (1 left + 1 right)
HP = 226          # padded height
NSLAB = 4         # row-slabs per image (56 rows each)
SR = 56
SLAB_ROWS = 58    # input rows per slab (56 + 2 halo)
SLAB_ELEMS = SLAB_ROWS * WP  # 13108
GP_SLAB = 14      # 4-row groups per slab
N_GROUPS = IMG_PC * NSLAB * GP_SLAB  # 112
MAGIC = 12582912.0  # 1.5 * 2**23 fp32 round-to-nearest-even trick
EPS = 1e-5
NTOT = float(16 * H * W)  # batch elements per channel (global)
HWs = H * W

_cache = {}
_last_ab = None     # filled by kernel(); test.py reuses them for timing B
_last_yr = None


def _build_nc(mode="final", loop_n=1, skip=()):
    assert mode in ("stats", "final")
    nc = bacc.Bacc("TRN2", target_bir_lowering=False, debug=False,
                   num_devices=N_CORES)
    if mode == "stats":
        xs_ext = nc.declare_dram_parameter("xs", [IMG_PC, 3, C, HP, WP], F16,
                                           isOutput=False)
        sw_ext = nc.declare_dram_parameter("sw", [96, 6, 128], F16,
                                           isOutput=False)
        sel1_ext = nc.declare_dram_parameter("sel1", [128, C], F32,
                                             isOutput=False)
        st_ext = nc.declare_dram_parameter("st", [C, 2], F32, isOutput=True)
        yr_ext = nc.declare_dram_parameter("yr", [4, 128, 28, 224], F16,
                                           isOutput=True)
        ab_ext = y_ext = None
    else:
        yr_ext = nc.declare_dram_parameter("yr", [4, 128, 28, 224], F16,
                                           isOutput=False)
        ab_ext = nc.declare_dram_parameter("ab", [128, 2], F32, isOutput=False)
        y_ext = nc.declare_dram_parameter("y", [IMG_PC, C, H, W], I8,
                                          isOutput=True)
        xs_ext = sw_ext = sel1_ext = st_ext = None

    with tile.TileContext(nc) as tc:
        with (
            tc.tile_pool(name="big", bufs=1) as big,
            tc.tile_pool(name="small", bufs=1) as small,
            tc.tile_pool(name="ph2", bufs=2) as ph2,
            tc.tile_pool(name="psum", bufs=1, space="PSUM") as psum,
        ):
            # y quarter-buffers: 28 groups each (half an image) so the
            # epilogue / writeback can stream at quarter granularity.
            yq = [big.tile([128, 28, 224], F16, name=f"yq{i}")
                  for i in range(4)]
            psum_t = psum.tile([128, 8, 512], F32)
            if mode == "stats":
                xb = [big.tile([96, SLAB_ELEMS], F16, name=f"xb{i}")
                      for i in range(2)]
                s_sb = small.tile([96, 6, 128], F16)
                stats_buf = small.tile([128, 56, 6], F32)
                sel1_sb = small.tile([128, C], F32)
                st_sb = small.tile([C, 2], F32)
                msq_scr = small.tile([128, 112], F32)
                red = small.tile([128, 4], F32)
                stats_sq = small.tile([128, 2], F32)
                nc.sync.dma_start(out=sel1_sb[:], in_=sel1_ext.ap())
                nc.sync.dma_start(out=s_sb[:], in_=sw_ext.ap())
            else:
                ab_sb = small.tile([128, 2], F32)
                nc.sync.dma_start(out=ab_sb[:], in_=ab_ext.ap())

            env = dict(locals())
            import contextlib
            loop_cm = (tc.For_i(0, loop_n, 1) if loop_n > 1
                       else contextlib.nullcontext())
            with loop_cm:
                if mode == "stats":
                    _body_stats(nc, tc, env, skip)
                else:
                    _body_final(nc, tc, env, skip)
    nc.compile()
    return nc


def _body_stats(nc, tc, env, skip=()):
    """Dispatch A: conv -> yq quarters -> yr HBM dump + bn stats -> st."""
    xb, yq, s_sb = env["xb"], env["yq"], env["s_sb"]
    stats_buf, psum_t = env["stats_buf"], env["psum_t"]
    xs_ap = env["xs_ext"].ap()
    yr_ap = env["yr_ext"].ap()

    for slab in range(IMG_PC * NSLAB):
        img, s = divmod(slab, NSLAB)
        x_c = xb[slab % 2]
        if "xdma" not in skip:
            src = bass.AP(
                tensor=xs_ap.tensor,
                offset=(xs_ap.offset + img * 3 * C * HP * WP
                        + SR * s * WP),
                ap=[[C * HP * WP, 3], [HP * WP, C], [1, SLAB_ELEMS]])
            nc.sync.dma_start(out=x_c[:], in_=src)
        for j in range(GP_SLAB if "mm" not in skip else 0):
            g_glob = slab * GP_SLAB + j
            bank = g_glob % 8
            hl = 4 * j
            for ai in range(6):
                a = ai - 1
                off = (hl + a + 1) * WP
                nc.tensor.matmul(
                    psum_t[0:128, bank, 0:224],
                    s_sb[0:96, ai, :],
                    x_c[0:96, off:off + 224],
                    start=(ai == 0), stop=(ai == 5))
            if g_glob % 2 == 1 and "drain" not in skip:
                # drain the (even, odd) bank pair in one ACT copy
                q, qg = divmod(g_glob - 1, 28)
                pair_src = psum_t[0:128, bank - 1:bank + 1, 0:224]
                nc.scalar.copy(yq[q][0:128, qg:qg + 2, :], pair_src)
                if "stats" not in skip:
                    pair = (g_glob - 1) // 2
                    nc.vector.bn_stats(
                        out=stats_buf[0:128, pair, :],
                        in_=yq[q][0:128, qg:qg + 2, :].rearrange(
                            "p a b -> p (a b)"))
        if slab % 2 == 1 and "ydma" not in skip and "drain" not in skip:
            q = slab // 2
            dst = bass.AP(
                tensor=yr_ap.tensor,
                offset=yr_ap.offset + q * 128 * 28 * 224,
                ap=[[28 * 224, 128], [1, 28 * 224]])
            nc.sync.dma_start(
                out=dst, in_=yq[q][:].rearrange("p a b -> p (a b)"))

    if "stats" not in skip and "drain" not in skip:
        _stats_reduce(nc, env)


def _body_final(nc, tc, env, skip=()):
    """Dispatch B: yr HBM -> yq -> affine+round+clip -> y NCHW bf16."""
    yq = env["yq"]
    yr_ap = env["yr_ext"].ap()
    for q in range(4):
        if "ydma" not in skip:
            src = bass.AP(
                tensor=yr_ap.tensor,
                offset=yr_ap.offset + q * 128 * 28 * 224,
                ap=[[28 * 224, 128], [1, 28 * 224]])
            nc.sync.dma_start(
                out=yq[q][:].rearrange("p a b -> p (a b)"), in_=src)
        if "ph2" not in skip:
            for ci in range(2):
                _phase2_chunk(nc, env, q, ci)


def _phase2_chunk(nc, env, q, ci):
    """Affine+round+clip 14 groups (a quarter-image half) and DMA out."""
    yq, ab_sb = env["yq"], env["ab_sb"]
    ph2 = env["ph2"]
    y_ap = env["y_ext"].ap()
    img, qi = divmod(q, 2)          # quarter q = image img, half qi
    ng = 14
    zin = yq[q][0:128, ng * ci:ng * (ci + 1), :].rearrange("p a b -> p (a b)")
    n = ng * 224
    u = ph2.tile([128, n], F32, tag="u")
    nc.scalar.activation(u[:], zin,
                         mybir.ActivationFunctionType.Identity,
                         bias=ab_sb[0:128, 1:2],
                         scale=ab_sb[0:128, 0:1])
    # v = 2*bn + 2; int8 store rounds RNE (verified on hw), so
    # int8(clip(v, 0, 4.5)) == clip(round(2*bn), -2, 2) + 2 exactly.
    o = ph2.tile([128, n], I8, tag="o")
    nc.vector.tensor_scalar(o[:], u[:], 0.0, 4.5,
                            mybir.AluOpType.max, mybir.AluOpType.min)
    # groups here are rows 4g+d, g in [28*qi+14*ci, +14), d = p//32
    g0 = 28 * qi + ng * ci
    for d in range(4):
        dst = bass.AP(
            tensor=y_ap.tensor,
            offset=y_ap.offset + img * C * HWs + (4 * g0 + d) * W,
            ap=[[HWs, C], [4 * W, ng], [1, W]])
        nc.sync.dma_start(out=dst, in_=o[32 * d:32 * d + 32, :])


def _stats_reduce(nc, env):
    """stats_buf [128,56,6] -> per-channel (sum, sumsq) [32,2] -> DRAM."""
    stats_buf, psum_t = env["stats_buf"], env["psum_t"]
    msq_scr, red, stats_sq = env["msq_scr"], env["red"], env["stats_sq"]
    sel1_sb, st_sb = env["sel1_sb"], env["st_sb"]
    st_ap = env["st_ext"].ap()

    stats_fl = stats_buf.rearrange("p s (e t) -> p (s e) t", e=2, t=3)
    means = stats_fl[:, :, 1]
    ctv = stats_fl[:, :, 2]
    nc.vector.tensor_reduce(red[:, 0:1], means, mybir.AxisListType.X,
                            mybir.AluOpType.add)
    nc.vector.tensor_tensor(msq_scr[:], means, means, mybir.AluOpType.mult)
    nc.vector.tensor_reduce(red[:, 1:2], msq_scr[:], mybir.AxisListType.X,
                            mybir.AluOpType.add)
    nc.vector.tensor_reduce(red[:, 2:3], ctv, mybir.AxisListType.X,
                            mybir.AluOpType.add)
    nc.vector.tensor_scalar_mul(stats_sq[:, 0:1], red[:, 0:1], 224.0)
    nc.vector.tensor_scalar_mul(red[:, 3:4], red[:, 1:2], 224.0)
    nc.vector.tensor_tensor(stats_sq[:, 1:2], red[:, 3:4], red[:, 2:3],
                            mybir.AluOpType.add)
    nc.tensor.matmul(psum_t[0:C, 0, 0:2], sel1_sb[:], stats_sq[:],
                     start=True, stop=True)
    nc.scalar.copy(st_sb[:], psum_t[0:C, 0, 0:2])
    nc.sync.dma_start(out=st_ap, in_=st_sb[:])


def _get_nc(**kw):
    kw.pop("collective", None)
    kw.setdefault("mode", "final")
    key = tuple(sorted((k, tuple(v) if isinstance(v, (list, tuple, set)) else v)
                       for k, v in kw.items()))
    if key not in _cache:
        _cache[key] = _build_nc(**kw)
    return _cache[key]


def _prep_x(x):
    """[16,32,224,224] f32 -> per-core [2,3,32,226,226] f16 shifted copies."""
    xq = np.asarray(x, dtype=np.float32).astype(ml_dtypes.float16
                    if hasattr(ml_dtypes, "float16") else np.float16)
    xp = np.zeros((16, C, HP, WP), dtype=xq.dtype)
    xp[:, :, 1:225, 1:225] = xq
    xs3 = np.zeros((16, 3, C, HP, WP), dtype=xq.dtype)
    xs3[:, 0] = xp
    xs3[:, 1, :, :, :WP - 1] = xp[:, :, :, 1:]
    xs3[:, 2, :, :, :WP - 2] = xp[:, :, :, 2:]
    return xs3


def _prep_w(weight):
    """OIHW weight -> lhsT stack sw[96, 6, 128] f16 (binarized)."""
    w_bin = np.where(np.asarray(weight, dtype=np.float32) >= 0, 1.0,
                     -1.0).astype(np.float32)
    sw = np.zeros((96, 6, 128), dtype=np.float32)
    for ai in range(6):
        a = ai - 1
        for d in range(4):
            kh = a + 1 - d
            if 0 <= kh <= 2:
                for kw in range(3):
                    # lhsT[32*kw+ci, ai, 32*d+co] = w_bin[co, ci, kh, kw]
                    sw[32 * kw:32 * kw + 32, ai, 32 * d:32 * d + 32] = \
                        w_bin[:, :, kh, kw].T
    return sw.astype(ml_dtypes.float16
                     if hasattr(ml_dtypes, "float16") else np.float16)


def _sel1():
    p = np.arange(128)
    return (p[:, None] % 32 == np.arange(C)[None, :]).astype(np.float32)


def make_in_maps_A(x, weight):
    xs3 = _prep_x(x)
    sw = _prep_w(weight)
    sel1 = _sel1()
    return [{"xs": xs3[IMG_PC * c:IMG_PC * (c + 1)], "sw": sw, "sel1": sel1}
            for c in range(N_CORES)]


def make_in_maps_B(yr_list, ab):
    return [{"yr": yr_list[c], "ab": ab} for c in range(N_CORES)]


def reduce_stats_host(st_list, gamma, beta):
    """8x [32,2] partials -> ab [128,2] = (2*scale, 2*bias+2) replicated."""
    st = np.sum(np.stack([np.asarray(s, np.float64) for s in st_list]), axis=0)
    mean = st[:, 0] / NTOT
    var = st[:, 1] / NTOT - mean * mean
    rsq = 1.0 / np.sqrt(var + EPS)
    g = np.asarray(gamma, np.float64)
    b = np.asarray(beta, np.float64)
    scale = g * rsq
    bias = b - mean * scale
    ab32 = np.stack([2.0 * scale, 2.0 * bias + 2.0], axis=1).astype(np.float32)
    return np.tile(ab32, (4, 1))    # [128, 2], p = 32d + co


def kernel(x, weight, gamma, beta):
    global _last_ab, _last_yr
    nc_a = _get_nc(mode="stats")
    in_a = make_in_maps_A(x, weight)
    res_a = run_bass_kernel_spmd(nc_a, in_a, list(range(N_CORES)))
    ab = reduce_stats_host([res_a.results[c]["st"] for c in range(N_CORES)],
                           gamma, beta)
    yr_list = [np.asarray(res_a.results[c]["yr"]) for c in range(N_CORES)]
    _last_ab, _last_yr = ab, yr_list

    nc_b = _get_nc(mode="final")
    in_b = make_in_maps_B(yr_list, ab)
    res_b = run_bass_kernel_spmd(nc_b, in_b, list(range(N_CORES)))
    out = np.concatenate([res_b.results[c]["y"] for c in range(N_CORES)],
                         axis=0)
    return (out.astype(np.float32) - 2.0) * 0.5


# revision 26
# speedup vs baseline: 4.4110x; 1.0816x over previous
"""Trainium2 Bass kernel for BinarizeConv2d block:
   y = round(2*clip(BN(conv3x3(x, sign(w))), -1, 1))/2

Data-parallel sharding: each of 8 cores convolves 2 images over ALL 32
output channels.  BN needs full-batch stats, and NRT collectives cannot sit
inside a For_i hardware loop in this environment, so the kernel runs as TWO
dispatches:
  A ("stats"): conv -> per-core (sum, sumsq) per channel [32,2]; the host
     adds the 8 cores' partials and derives the BN scale/bias (tiny math).
  B ("final"): conv again (cheaper than a fp32 y round-trip through HBM),
     then fused affine+round(magic)+clip -> bf16 {0..4}; host maps to
     {-1,-.5,0,.5,1}.

Conv lowering (the perf core of this kernel): with 32 output channels per
core we can use M=128 PE columns = 32co x 4 row-phases.  rhs partitions
hold K=96 = 32ci x 3 column-shifted copies of x (kw baked into partition
groups), and 6 matmuls with row-offset rhs (a = -1..4) accumulate a
[128, 224] psum tile covering FOUR output rows: column (32d+co) gets
weights W[co,ci,a+1-d,kw] (zero when a+1-d is outside 0..2).  That is
6 matmuls x 224 cycles per 4 rows versus the old channel-sharded scheme's
~63 tiny matmuls per 2 rows -- ~8x fewer PE cycles and ~24x fewer
instructions (the old kernel was instruction-issue-bound).
"""
import sys
sys.path.insert(0, "/opt/trn_rl_repo")
import numpy as np
import ml_dtypes
import concourse.bass as bass
import concourse.bacc as bacc
import concourse.tile as tile
from concourse import mybir
from concourse.bass_utils import run_bass_kernel_spmd

F32 = mybir.dt.float32
F16 = mybir.dt.float16
BF16 = mybir.dt.bfloat16
I8 = mybir.dt.int8

N_CORES = 8
IMG_PC = 2        # images per core
C = 32
H = W = 224
WP = 226          # padded width (1 left + 1 right)
HP = 226          # padded height
NSLAB = 4         # row-slabs per image (56 rows each)
SR = 56
SLAB_ROWS = 58    # input rows per slab (56 + 2 halo)
SLAB_ELEMS = SLAB_ROWS * WP  # 13108
GP_SLAB = 14      # 4-row groups per slab
N_GROUPS = IMG_PC * NSLAB * GP_SLAB  # 112
MAGIC = 12582912.0  # 1.5 * 2**23 fp32 round-to-nearest-even trick
EPS = 1e-5
NTOT = float(16 * H * W)  # batch elements per channel (global)
HWs = H * W

_cache = {}
_last_ab = None     # filled by kernel(); test.py reuses them for timing B
_last_yr = None


def _build_nc(mode="final", loop_n=1, skip=(), stagger=False):
    assert mode in ("stats", "final")
    nc = bacc.Bacc("TRN2", target_bir_lowering=False, debug=False,
                   num_devices=N_CORES)
    if mode == "stats":
        xs_ext = nc.declare_dram_parameter("xs", [IMG_PC, 3, C, HP, WP], F16,
                                           isOutput=False)
        sw_ext = nc.declare_dram_parameter("sw", [96, 6, 128], F16,
                                           isOutput=False)
        sel1_ext = nc.declare_dram_parameter("sel1", [128, C], F32,
                                             isOutput=False)
        st_ext = nc.declare_dram_parameter("st", [C, 2], F32, isOutput=True)
        yr_ext = nc.declare_dram_parameter("yr", [8, 128, 14, 224], F16,
                                           isOutput=True)
        ab_ext = y_ext = None
    else:
        yr_ext = nc.declare_dram_parameter("yr", [8, 128, 14, 224], F16,
                                           isOutput=False)
        ab_ext = nc.declare_dram_parameter("ab", [128, 2], F32, isOutput=False)
        y_ext = nc.declare_dram_parameter("y", [IMG_PC, C, H, W], I8,
                                          isOutput=True)
        xs_ext = sw_ext = sel1_ext = st_ext = None

    with tile.TileContext(nc) as tc:
        with (
            tc.tile_pool(name="big", bufs=1) as big,
            tc.tile_pool(name="small", bufs=1) as small,
            tc.tile_pool(name="ph2", bufs=2) as ph2,
            tc.tile_pool(name="psum", bufs=1, space="PSUM") as psum,
        ):
            # y eighth-buffers: 14 groups each (quarter image) so the
            # epilogue / writeback can stream at eighth granularity.
            yq = [big.tile([128, 14, 224], F16, name=f"yq{i}")
                  for i in range(8)]
            psum_t = psum.tile([128, 8, 512], F32)
            if mode == "stats":
                xb = [big.tile([96, SLAB_ELEMS], F16, name=f"xb{i}")
                      for i in range(2)]
                s_sb = small.tile([96, 6, 128], F16)
                stats_buf = small.tile([128, 56, 6], F32)
                sel1_sb = small.tile([128, C], F32)
                st_sb = small.tile([C, 2], F32)
                msq_scr = small.tile([128, 112], F32)
                red = small.tile([128, 4], F32)
                stats_sq = small.tile([128, 2], F32)
                nc.sync.dma_start(out=sel1_sb[:], in_=sel1_ext.ap())
                nc.sync.dma_start(out=s_sb[:], in_=sw_ext.ap())
            else:
                ab_sb = small.tile([128, 2], F32)
                nc.sync.dma_start(out=ab_sb[:], in_=ab_ext.ap())

            env = dict(locals())
            import contextlib
            loop_cm = (tc.For_i(0, loop_n, 1, staggered_reset=stagger)
                       if loop_n > 1 else contextlib.nullcontext())
            with loop_cm:
                if mode == "stats":
                    _body_stats(nc, tc, env, skip)
                else:
                    _body_final(nc, tc, env, skip)
    nc.compile()
    return nc


def _body_stats(nc, tc, env, skip=()):
    """Dispatch A: conv -> yq quarters -> yr HBM dump + bn stats -> st."""
    xb, yq, s_sb = env["xb"], env["yq"], env["s_sb"]
    stats_buf, psum_t = env["stats_buf"], env["psum_t"]
    xs_ap = env["xs_ext"].ap()
    yr_ap = env["yr_ext"].ap()

    for slab in range(IMG_PC * NSLAB):
        img, s = divmod(slab, NSLAB)
        x_c = xb[slab % 2]
        if "xdma" not in skip:
            src = bass.AP(
                tensor=xs_ap.tensor,
                offset=(xs_ap.offset + img * 3 * C * HP * WP
                        + SR * s * WP),
                ap=[[C * HP * WP, 3], [HP * WP, C], [1, SLAB_ELEMS]])
            nc.sync.dma_start(out=x_c[:], in_=src)
        for j in range(GP_SLAB if "mm" not in skip else 0):
            g_glob = slab * GP_SLAB + j
            bank = g_glob % 8
            hl = 4 * j
            for ai in range(6):
                a = ai - 1
                off = (hl + a + 1) * WP
                nc.tensor.matmul(
                    psum_t[0:128, bank, 0:224],
                    s_sb[0:96, ai, :],
                    x_c[0:96, off:off + 224],
                    start=(ai == 0), stop=(ai == 5))
            if g_glob % 2 == 1 and "drain" not in skip:
                # drain the (even, odd) bank pair in one ACT copy
                e, qg = divmod(g_glob - 1, 14)
                pair_src = psum_t[0:128, bank - 1:bank + 1, 0:224]
                nc.scalar.copy(yq[e][0:128, qg:qg + 2, :], pair_src)
                if "stats" not in skip:
                    pair = (g_glob - 1) // 2
                    nc.vector.bn_stats(
                        out=stats_buf[0:128, pair, :],
                        in_=yq[e][0:128, qg:qg + 2, :].rearrange(
                            "p a b -> p (a b)"))
                if qg == 12 and "ydma" not in skip:
                    # eighth e complete: stream it to HBM via SWDGE
                    dst = bass.AP(
                        tensor=yr_ap.tensor,
                        offset=yr_ap.offset + e * 128 * 14 * 224,
                        ap=[[14 * 224, 128], [1, 14 * 224]])
                    nc.gpsimd.dma_start(
                        out=dst, in_=yq[e][:].rearrange("p a b -> p (a b)"))

    if "stats" not in skip and "drain" not in skip:
        _stats_reduce(nc, env)


def _body_final(nc, tc, env, skip=()):
    """Dispatch B: yr HBM -> yq -> affine+round+clip -> y NCHW int8."""
    yq = env["yq"]
    yr_ap = env["yr_ext"].ap()
    for e in range(8):
        if "ydma" not in skip:
            src = bass.AP(
                tensor=yr_ap.tensor,
                offset=yr_ap.offset + e * 128 * 14 * 224,
                ap=[[14 * 224, 128], [1, 14 * 224]])
            nc.sync.dma_start(
                out=yq[e][:].rearrange("p a b -> p (a b)"), in_=src)
        if "ph2" not in skip:
            _phase2_chunk(nc, env, e)


def _phase2_chunk(nc, env, e):
    """Affine+round+clip one eighth (14 groups = 56 rows) and DMA out."""
    yq, ab_sb = env["yq"], env["ab_sb"]
    ph2 = env["ph2"]
    y_ap = env["y_ext"].ap()
    img, ei = divmod(e, 4)          # eighth e = image img, slab ei
    ng = 14
    zin = yq[e][:].rearrange("p a b -> p (a b)")
    n = ng * 224
    u = ph2.tile([128, n], F32, tag="u")
    nc.scalar.activation(u[:], zin,
                         mybir.ActivationFunctionType.Identity,
                         bias=ab_sb[0:128, 1:2],
                         scale=ab_sb[0:128, 0:1])
    # v = 2*bn + 2; int8 store rounds RNE (verified on hw), so
    # int8(clip(v, 0, 4.5)) == clip(round(2*bn), -2, 2) + 2 exactly.
    o = ph2.tile([128, n], I8, tag="o")
    nc.vector.tensor_scalar(o[:], u[:], 0.0, 4.5,
                            mybir.AluOpType.max, mybir.AluOpType.min)
    # groups here are rows 4g+d, g in [14*ei, +14), d = p//32
    g0 = ng * ei
    for d in range(4):
        dst = bass.AP(
            tensor=y_ap.tensor,
            offset=y_ap.offset + img * C * HWs + (4 * g0 + d) * W,
            ap=[[HWs, C], [4 * W, ng], [1, W]])
        nc.gpsimd.dma_start(out=dst, in_=o[32 * d:32 * d + 32, :])


def _stats_reduce(nc, env):
    """stats_buf [128,56,6] -> per-channel (sum, sumsq) [32,2] -> DRAM."""
    stats_buf, psum_t = env["stats_buf"], env["psum_t"]
    msq_scr, red, stats_sq = env["msq_scr"], env["red"], env["stats_sq"]
    sel1_sb, st_sb = env["sel1_sb"], env["st_sb"]
    st_ap = env["st_ext"].ap()

    stats_fl = stats_buf.rearrange("p s (e t) -> p (s e) t", e=2, t=3)
    means = stats_fl[:, :, 1]
    ctv = stats_fl[:, :, 2]
    nc.vector.tensor_reduce(red[:, 0:1], means, mybir.AxisListType.X,
                            mybir.AluOpType.add)
    nc.vector.tensor_tensor(msq_scr[:], means, means, mybir.AluOpType.mult)
    nc.vector.tensor_reduce(red[:, 1:2], msq_scr[:], mybir.AxisListType.X,
                            mybir.AluOpType.add)
    nc.vector.tensor_reduce(red[:, 2:3], ctv, mybir.AxisListType.X,
                            mybir.AluOpType.add)
    nc.vector.tensor_scalar_mul(stats_sq[:, 0:1], red[:, 0:1], 224.0)
    nc.vector.tensor_scalar_mul(red[:, 3:4], red[:, 1:2], 224.0)
    nc.vector.tensor_tensor(stats_sq[:, 1:2], red[:, 3:4], red[:, 2:3],
                            mybir.AluOpType.add)
    nc.tensor.matmul(psum_t[0:C, 0, 0:2], sel1_sb[:], stats_sq[:],
                     start=True, stop=True)
    nc.scalar.copy(st_sb[:], psum_t[0:C, 0, 0:2])
    nc.sync.dma_start(out=st_ap, in_=st_sb[:])


def _get_nc(**kw):
    kw.pop("collective", None)
    kw.setdefault("mode", "final")
    key = tuple(sorted((k, tuple(v) if isinstance(v, (list, tuple, set)) else v)
                       for k, v in kw.items()))
    if key not in _cache:
        _cache[key] = _build_nc(**kw)
    return _cache[key]


def _prep_x(x):
    """[16,32,224,224] f32 -> per-core [2,3,32,226,226] f16 shifted copies."""
    xq = np.asarray(x, dtype=np.float32).astype(ml_dtypes.float16
                    if hasattr(ml_dtypes, "float16") else np.float16)
    xp = np.zeros((16, C, HP, WP), dtype=xq.dtype)
    xp[:, :, 1:225, 1:225] = xq
    xs3 = np.zeros((16, 3, C, HP, WP), dtype=xq.dtype)
    xs3[:, 0] = xp
    xs3[:, 1, :, :, :WP - 1] = xp[:, :, :, 1:]
    xs3[:, 2, :, :, :WP - 2] = xp[:, :, :, 2:]
    return xs3


def _prep_w(weight):
    """OIHW weight -> lhsT stack sw[96, 6, 128] f16 (binarized)."""
    w_bin = np.where(np.asarray(weight, dtype=np.float32) >= 0, 1.0,
                     -1.0).astype(np.float32)
    sw = np.zeros((96, 6, 128), dtype=np.float32)
    for ai in range(6):
        a = ai - 1
        for d in range(4):
            kh = a + 1 - d
            if 0 <= kh <= 2:
                for kw in range(3):
                    # lhsT[32*kw+ci, ai, 32*d+co] = w_bin[co, ci, kh, kw]
                    sw[32 * kw:32 * kw + 32, ai, 32 * d:32 * d + 32] = \
                        w_bin[:, :, kh, kw].T
    return sw.astype(ml_dtypes.float16
                     if hasattr(ml_dtypes, "float16") else np.float16)


def _sel1():
    p = np.arange(128)
    return (p[:, None] % 32 == np.arange(C)[None, :]).astype(np.float32)


def make_in_maps_A(x, weight):
    xs3 = _prep_x(x)
    sw = _prep_w(weight)
    sel1 = _sel1()
    return [{"xs": xs3[IMG_PC * c:IMG_PC * (c + 1)], "sw": sw, "sel1": sel1}
            for c in range(N_CORES)]


def make_in_maps_B(yr_list, ab):
    return [{"yr": yr_list[c], "ab": ab} for c in range(N_CORES)]


def reduce_stats_host(st_list, gamma, beta):
    """8x [32,2] partials -> ab [128,2] = (2*scale, 2*bias+2) replicated."""
    st = np.sum(np.stack([np.asarray(s, np.float64) for s in st_list]), axis=0)
    mean = st[:, 0] / NTOT
    var = st[:, 1] / NTOT - mean * mean
    rsq = 1.0 / np.sqrt(var + EPS)
    g = np.asarray(gamma, np.float64)
    b = np.asarray(beta, np.float64)
    scale = g * rsq
    bias = b - mean * scale
    ab32 = np.stack([2.0 * scale, 2.0 * bias + 2.0], axis=1).astype(np.float32)
    return np.tile(ab32, (4, 1))    # [128, 2], p = 32d + co


def kernel(x, weight, gamma, beta):
    global _last_ab, _last_yr
    nc_a = _get_nc(mode="stats")
    in_a = make_in_maps_A(x, weight)
    res_a = run_bass_kernel_spmd(nc_a, in_a, list(range(N_CORES)))
    ab = reduce_stats_host([res_a.results[c]["st"] for c in range(N_CORES)],
                           gamma, beta)
    yr_list = [np.asarray(res_a.results[c]["yr"]) for c in range(N_CORES)]
    _last_ab, _last_yr = ab, yr_list

    nc_b = _get_nc(mode="final")
    in_b = make_in_maps_B(yr_list, ab)
    res_b = run_bass_kernel_spmd(nc_b, in_b, list(range(N_CORES)))
    out = np.concatenate([res_b.results[c]["y"] for c in range(N_CORES)],
                         axis=0)
    return (out.astype(np.float32) - 2.0) * 0.5
